# revision 2
# baseline (speedup 1.0000x reference)
"""EdgeConv (GNN message passing) Trainium2 Bass kernel, 8-core SPMD.

Strategy (destination / node sharding — no collectives needed):
  * Core r owns destination-node range [r*12500, (r+1)*12500) and ALL edges
    whose col (destination) lands in that range.  x and MLP params are
    replicated, so the x[row] gather cost is the same as edge-sharding, but
    the segment-sum aggregation is core-local: no all-reduce at all.
  * Within a core, edges are sorted by destination and grouped into 98
    blocks of 128 destination nodes.  Per block the aggregation becomes a
    sequence of 128x128 one-hot matmuls accumulated in PSUM (transposed:
    aggT += h_chunk.T @ S), with one-hot S built on-device from iota +
    is_equal against the (col % 128) value of each edge.
  * Block edge counts are padded to a static per-block-slot maximum across
    cores so a single SPMD program serves all 8 cores.  Pad edges carry
    col_rel = 128 -> all-zero one-hot row -> contribute nothing.
  * msg MLP: h = [x[row] | edge_attr | 1] @ [W1; b1] (bias folded as a
    ones row shipped with edge_attr.T), LayerNorm via bn_stats/bn_aggr,
    exact GELU fused with the LN affine into one ScalarE activation
    (gelu(h*r - mu*r)).
  * W2 and the aggregation never materialize `agg`: since agg only feeds
    the update MLP, fold W2u = W2 @ Wu[128:] and the per-node edge count
    carries b2:  u_pre = x@Wu[:128] + aggT.T@W2u + [counts|1]@[b2@Wub; bu].
  * Update LN+GELU per 128-node block, residual +x, store.  Host
    concatenates the 8 per-core [12500,128] slices.
"""
import math
import os
import time
from contextlib import ExitStack

import numpy as np

import concourse.bass as bass
import concourse.bacc as bacc
import concourse.mybir as mybir
import concourse.tile as tile
from concourse.bass_utils import run_bass_kernel_spmd
from concourse.masks import make_identity

# problem constants (hardcoded per harness contract)
N_NODES = 100000
N_EDGES = 1600000
F = 128          # node feature dim (IN_DIM == OUT_DIM)
EDGE_DIM = 32
HID = 256
EPS = 1e-5
N_CORES = 8
NPC = N_NODES // N_CORES          # 12500 nodes per core
P = 128
N_BLOCKS = math.ceil(NPC / P)     # 98
NODE_PAD = N_BLOCKS * P           # 12544
EA_CHUNK = 8                      # edge-attr tiles per DMA

f32 = mybir.dt.float32
i32 = mybir.dt.int32


def _preprocess(x, edge_index, edge_attr):
    """Sort/shard/pad edges by destination. Returns shared block layout and
    per-core input arrays."""
    row = np.ascontiguousarray(edge_index[0]).astype(np.int64)
    col = np.ascontiguousarray(edge_index[1]).astype(np.int64)

    order = np.argsort(col, kind="stable")
    col_s = col[order]

    # boundaries for (core, block)
    counts = np.zeros((N_CORES, N_BLOCKS), np.int64)
    los = np.zeros((N_CORES, N_BLOCKS), np.int64)
    for r in range(N_CORES):
        base = r * NPC
        for j in range(N_BLOCKS):
            lo = np.searchsorted(col_s, base + j * P, side="left")
            hi = np.searchsorted(col_s, base + min((j + 1) * P, NPC), side="left")
            los[r, j], counts[r, j] = lo, hi - lo

    bmax = counts.max(axis=0)
    n_tiles = np.maximum(1, np.ceil(bmax / P).astype(np.int64))   # tiles per block
    Bj = n_tiles * P
    tile_off = np.concatenate([[0], np.cumsum(n_tiles)])          # tile index offsets
    Ep = int(Bj.sum())
    T_total = Ep // P

    x_pad = np.zeros((N_CORES * NODE_PAD - (N_CORES - 1) * NPC if False else NPC * (N_CORES - 1) + NODE_PAD, F), np.float32)
    x_pad = np.zeros((NPC * (N_CORES - 1) + NODE_PAD, F), np.float32)
    x_pad[:N_NODES] = x

    ea32 = np.ascontiguousarray(edge_attr, dtype=np.float32)
    node_counts = np.bincount(col, minlength=N_NODES).astype(np.float32)

    per_core = []
    for r in range(N_CORES):
        asm = np.zeros(Ep, np.int64)
        valid = np.zeros(Ep, bool)
        for j in range(N_BLOCKS):
            lo, c = los[r, j], counts[r, j]
            dst = int(Bj[:j].sum())
            asm[dst:dst + c] = order[lo:lo + c]
            valid[dst:dst + c] = True

        row_pad = np.where(valid, row[asm], 0).astype(np.int32)
        colrel = np.where(valid, (col[asm] - r * NPC) % P, P).astype(np.float32)
        ea_pad = ea32[asm] * valid[:, None]

        row_tiles = np.ascontiguousarray(row_pad.reshape(T_total, P).T)          # [128, T]
        col_tiles = np.ascontiguousarray(colrel.reshape(T_total, P).T)           # [128, T] f32
        ea_t = np.concatenate([ea_pad.T, np.ones((1, Ep), np.float32)], 0)
        ea_t = np.ascontiguousarray(ea_t, dtype=np.float32)                      # [33, Ep]

        x_shard = np.ascontiguousarray(x_pad[r * NPC: r * NPC + NODE_PAD])       # [12544,128]
        xnt = np.ascontiguousarray(
            x_shard.reshape(N_BLOCKS, P, F).transpose(0, 2, 1))                  # [98,128,128]
        cnt = np.zeros(NODE_PAD, np.float32)
        cnt[:NPC] = node_counts[r * NPC:(r + 1) * NPC]
        cnt1 = np.ascontiguousarray(np.stack([cnt, np.ones(NODE_PAD, np.float32)]))  # [2,12544]

        per_core.append(dict(row_tiles=row_tiles, col_tiles=col_tiles, ea_t=ea_t,
                             x_shard=x_shard, xnt=xnt, cnt1=cnt1))

    return x_pad, per_core, n_tiles.tolist(), tile_off.tolist(), T_total, Ep


def _build_program(n_tiles, tile_off, T_total, Ep):
    nc = bacc.Bacc("TRN2", target_bir_lowering=False, debug=False,
                   num_devices=N_CORES)

    xpad_rows = NPC * (N_CORES - 1) + NODE_PAD
    x_d = nc.dram_tensor("x_pad", [xpad_rows, F], f32, kind="ExternalInput")
    xs_d = nc.dram_tensor("x_shard", [NODE_PAD, F], f32, kind="ExternalInput")
    xnt_d = nc.dram_tensor("xnt", [N_BLOCKS, F, P], f32, kind="ExternalInput")
    row_d = nc.dram_tensor("row_tiles", [P, T_total], i32, kind="ExternalInput")
    col_d = nc.dram_tensor("col_tiles", [P, T_total], f32, kind="ExternalInput")
    ea_d = nc.dram_tensor("ea_t", [EDGE_DIM + 1, Ep], f32, kind="ExternalInput")
    cnt_d = nc.dram_tensor("cnt1", [2, NODE_PAD], f32, kind="ExternalInput")
    w1a_d = nc.dram_tensor("w1a", [F, HID], f32, kind="ExternalInput")
    w1b_d = nc.dram_tensor("w1b", [EDGE_DIM + 1, HID], f32, kind="ExternalInput")
    wua_d = nc.dram_tensor("wua", [F, F], f32, kind="ExternalInput")
    w2u_d = nc.dram_tensor("w2u", [P, HID], f32, kind="ExternalInput")
    bb_d = nc.dram_tensor("b2ubu", [2, F], f32, kind="ExternalInput")
    out_d = nc.dram_tensor("out", [NODE_PAD, F], f32, kind="ExternalOutput")

    with tile.TileContext(nc) as tc, ExitStack() as ctx:
        cb = ctx.enter_context(tc.tile_pool(name="cb", bufs=1))
        eap = ctx.enter_context(tc.tile_pool(name="eap", bufs=3))
        gp = ctx.enter_context(tc.tile_pool(name="gp", bufs=6))
        gts = ctx.enter_context(tc.tile_pool(name="gts", bufs=3))
        hsp = ctx.enter_context(tc.tile_pool(name="hsp", bufs=3))
        sp = ctx.enter_context(tc.tile_pool(name="sp", bufs=3))
        stp = ctx.enter_context(tc.tile_pool(name="stp", bufs=4))
        blk = ctx.enter_context(tc.tile_pool(name="blk", bufs=2))
        ps_gt = ctx.enter_context(tc.tile_pool(name="ps_gt", bufs=2, space="PSUM"))
        ps_h = ctx.enter_context(tc.tile_pool(name="ps_h", bufs=2, space="PSUM"))
        ps_agg = ctx.enter_context(tc.tile_pool(name="ps_agg", bufs=1, space="PSUM"))
        ps_u = ctx.enter_context(tc.tile_pool(name="ps_u", bufs=2, space="PSUM"))

        ident = cb.tile([P, P], f32)
        make_identity(nc, ident[:])
        iota = cb.tile([P, P], f32)
        nc.gpsimd.iota(iota[:], pattern=[[1, P]], base=0, channel_multiplier=0,
                       allow_small_or_imprecise_dtypes=True)
        epsb = cb.tile([P, 1], f32)
        nc.vector.memset(epsb[:], EPS)
        w1a_s = cb.tile([F, HID], f32)
        nc.sync.dma_start(w1a_s[:], w1a_d.ap())
        w1b_s = cb.tile([EDGE_DIM + 1, HID], f32)
        nc.sync.dma_start(w1b_s[:], w1b_d.ap())
        wua_s = cb.tile([F, F], f32)
        nc.sync.dma_start(wua_s[:], wua_d.ap())
        w2u_s = cb.tile([P, HID], f32)
        nc.sync.dma_start(w2u_s[:], w2u_d.ap())
        bb_s = cb.tile([2, F], f32)
        nc.sync.dma_start(bb_s[:], bb_d.ap())
        row_all = cb.tile([P, T_total], i32)
        nc.sync.dma_start(row_all[:], row_d.ap())
        col_all = cb.tile([P, T_total], f32)
        nc.sync.dma_start(col_all[:], col_d.ap())
        cnt_s = cb.tile([2, NODE_PAD], f32)
        nc.sync.dma_start(cnt_s[:], cnt_d.ap())

        ea_tile = None
        for j in range(N_BLOCKS):
            nt = n_tiles[j]
            agg0 = ps_agg.tile([P, P], f32, space="PSUM", tag="agg0")
            agg1 = ps_agg.tile([P, P], f32, space="PSUM", tag="agg1")
            for ti in range(nt):
                t = tile_off[j] + ti
                if t % EA_CHUNK == 0:
                    w = min(EA_CHUNK, T_total - t) * P
                    ea_tile = eap.tile([EDGE_DIM + 1, EA_CHUNK * P], f32, tag="ea")
                    nc.sync.dma_start(ea_tile[:, :w], ea_d.ap()[:, t * P: t * P + w])
                ei = (t % EA_CHUNK) * P

                g = gp.tile([P, F], f32, tag="g")
                nc.gpsimd.indirect_dma_start(
                    out=g[:], out_offset=None, in_=x_d.ap(),
                    in_offset=bass.IndirectOffsetOnAxis(ap=row_all[:, t:t + 1], axis=0))

                gt_ps = ps_gt.tile([P, F], f32, space="PSUM", tag="gtp")
                nc.tensor.transpose(out=gt_ps[:], in_=g[:], identity=ident[:])
                gt = gts.tile([P, F], f32, tag="gt")
                nc.scalar.copy(gt[:], gt_ps[:])

                h_ps = ps_h.tile([P, HID], f32, space="PSUM", tag="h")
                nc.tensor.matmul(out=h_ps[:], lhsT=gt[:], rhs=w1a_s[:],
                                 start=True, stop=False)
                nc.tensor.matmul(out=h_ps[:], lhsT=ea_tile[:, ei:ei + P],
                                 rhs=w1b_s[:], start=False, stop=True)

                st = stp.tile([P, 6], f32, tag="st")
                nc.vector.bn_stats(st[:], h_ps[:])
                mv = stp.tile([P, 2], f32, tag="mv")
                nc.vector.bn_aggr(mv[:], st[:])
                sd = stp.tile([P, 1], f32, tag="sd")
                nc.scalar.activation(sd[:], mv[:, 1:2],
                                     mybir.ActivationFunctionType.Sqrt,
                                     bias=epsb[:, 0:1], scale=1.0)
                r = stp.tile([P, 1], f32, tag="r")
                nc.vector.reciprocal(r[:], sd[:])
                nmr = stp.tile([P, 1], f32, tag="nmr")
                nc.vector.tensor_scalar(nmr[:], mv[:, 0:1], r[:, 0:1], -1.0,
                                        mybir.AluOpType.mult, mybir.AluOpType.mult)

                hs = hsp.tile([P, HID], f32, tag="hs")
                nc.scalar.activation(hs[:], h_ps[:],
                                     mybir.ActivationFunctionType.Gelu,
                                     bias=nmr[:, 0:1], scale=r[:, 0:1])

                S = sp.tile([P, P], f32, tag="S")
                nc.vector.tensor_scalar(S[:], iota[:], col_all[:, t:t + 1], None,
                                        mybir.AluOpType.is_equal)

                nc.tensor.matmul(out=agg0[:], lhsT=hs[:, 0:P], rhs=S[:],
                                 start=(ti == 0), stop=(ti == nt - 1))
                nc.tensor.matmul(out=agg1[:], lhsT=hs[:, P:HID], rhs=S[:],
                                 start=(ti == 0), stop=(ti == nt - 1))

            # ---- per-block update MLP ----
            aggt = blk.tile([P, HID], f32, tag="aggt")
            nc.scalar.copy(aggt[:, 0:P], agg0[:])
            nc.scalar.copy(aggt[:, P:HID], agg1[:])

            xnt_s = blk.tile([F, P], f32, tag="xnt")
            nc.sync.dma_start(xnt_s[:], xnt_d.ap()[j])

            u_ps = ps_u.tile([P, F], f32, space="PSUM", tag="u")
            nc.tensor.matmul(out=u_ps[:], lhsT=xnt_s[:], rhs=wua_s[:],
                             start=True, stop=False)
            nc.tensor.matmul(out=u_ps[:], lhsT=aggt[:, 0:P], rhs=w2u_s[:, 0:P],
                             start=False, stop=False)
            nc.tensor.matmul(out=u_ps[:], lhsT=aggt[:, P:HID], rhs=w2u_s[:, P:HID],
                             start=False, stop=False)
            nc.tensor.matmul(out=u_ps[:], lhsT=cnt_s[:, j * P:(j + 1) * P],
                             rhs=bb_s[:], start=False, stop=True)

            stu = stp.tile([P, 6], f32, tag="stu")
            nc.vector.bn_stats(stu[:], u_ps[:])
            mvu = stp.tile([P, 2], f32, tag="mvu")
            nc.vector.bn_aggr(mvu[:], stu[:])
            sdu = stp.tile([P, 1], f32, tag="sdu")
            nc.scalar.activation(sdu[:], mvu[:, 1:2],
                                 mybir.ActivationFunctionType.Sqrt,
                                 bias=epsb[:, 0:1], scale=1.0)
            ru = stp.tile([P, 1], f32, tag="ru")
            nc.vector.reciprocal(ru[:], sdu[:])
            nmru = stp.tile([P, 1], f32, tag="nmru")
            nc.vector.tensor_scalar(nmru[:], mvu[:, 0:1], ru[:, 0:1], -1.0,
                                    mybir.AluOpType.mult, mybir.AluOpType.mult)

            us = blk.tile([P, F], f32, tag="us")
            nc.scalar.activation(us[:], u_ps[:],
                                 mybir.ActivationFunctionType.Gelu,
                                 bias=nmru[:, 0:1], scale=ru[:, 0:1])

            xn_s = blk.tile([P, F], f32, tag="xn")
            nc.sync.dma_start(xn_s[:], xs_d.ap()[j * P:(j + 1) * P, :])
            uo = blk.tile([P, F], f32, tag="uo")
            nc.vector.tensor_tensor(out=uo[:], in0=us[:], in1=xn_s[:],
                                    op=mybir.AluOpType.add)
            nc.sync.dma_start(out_d.ap()[j * P:(j + 1) * P, :], uo[:])

    nc.compile()
    return nc


def run(inputs, trace=False, tmpdir=None):
    x = np.asarray(inputs["x"], np.float32)
    W1 = np.asarray(inputs["W1"], np.float32)
    b1 = np.asarray(inputs["b1"], np.float32)
    g1 = np.asarray(inputs["g1"], np.float32)
    be1 = np.asarray(inputs["be1"], np.float32)
    W2 = np.asarray(inputs["W2"], np.float32)
    b2 = np.asarray(inputs["b2"], np.float32)
    Wu = np.asarray(inputs["Wu"], np.float32)
    bu = np.asarray(inputs["bu"], np.float32)
    gu = np.asarray(inputs["gu"], np.float32)
    beu = np.asarray(inputs["beu"], np.float32)

    if not (np.all(g1 == 1) and np.all(be1 == 0) and np.all(gu == 1)
            and np.all(beu == 0)):
        raise NotImplementedError("nontrivial LayerNorm affine not supported")

    t0 = time.time()
    x_pad, per_core, n_tiles, tile_off, T_total, Ep = _preprocess(
        x, inputs["edge_index"], inputs["edge_attr"])

    # folded weights (shared across cores)
    w1a = np.ascontiguousarray(W1[:F])                                    # [128,256]
    w1b = np.ascontiguousarray(np.concatenate([W1[F:], b1[None, :]], 0))  # [33,256]
    wua = np.ascontiguousarray(Wu[:F])                                    # [128,128]
    wub = Wu[F:]                                                          # [128,128]
    W2u = (W2 @ wub).astype(np.float32)                                   # [256,128]
    w2u = np.ascontiguousarray(
        W2u.reshape(2, P, F).transpose(1, 0, 2).reshape(P, 2 * F))        # [128,256]
    b2ubu = np.ascontiguousarray(np.stack([b2 @ wub, bu]))                # [2,128]

    shared = dict(x_pad=x_pad, w1a=w1a, w1b=w1b, wua=wua, w2u=w2u, b2ubu=b2ubu)
    in_maps = [{**shared, **pc} for pc in per_core]
    t1 = time.time()

    nc = _build_program(n_tiles, tile_off, T_total, Ep)
    t2 = time.time()

    res = run_bass_kernel_spmd(nc, in_maps, core_ids=list(range(N_CORES)),
                               trace=trace, tmpdir=tmpdir,
                               trace_cores=[0] if trace else None)
    t3 = time.time()
    if os.environ.get("KERNEL_VERBOSE"):
        print(f"preprocess {t1-t0:.1f}s  build+compile {t2-t1:.1f}s  run {t3-t2:.1f}s")

    out = np.concatenate([res.results[r]["out"][:NPC] for r in range(N_CORES)], 0)
    return out, res


def kernel(**inputs):
    out, _ = run(inputs, trace=False)
    return out
